# revision 1
# baseline (speedup 1.0000x reference)
"""DiceLoss kernel for Trainium2 (8 NeuronCores, batch-sharded).

Math per image (H=W=1024):
    s  = sigmoid(output)
    P  = avgpool31x31(target)            (zero-padded box sum / 961)
    w  = 1 + 5*|P - target|
    inter = sum(s*t*w);  mask = sum((t+s)*w)
    loss  = 1 - (2*inter + 1e-3) / (mask + 1e-3);  final = mean over batch

Device strategy (per core: 4 images, 8 row-tiles of [128, 1024] each,
processed in pairs of row-tiles = [128, 2048] working set):
  - horizontal 31-tap box: ONE tensor_tensor_scan per row-tile
        state = (t[x+15] + state) - t[x-16]   over a zero-padded buffer
  - vertical 31-tap box + the "- 961*t" subtraction: PE band matmuls
        psum = Bdiag@H_i + Babove@H_{i-1} + Bbelow@H_{i+1} - 961*I@t_i
  - w1 = |5/961 * psum| on ACT (free affine before Abs), bf16 out
  - products on DVE in bf16 (2x mode): u = t*(1+w1), v = s*(1+w1), p2 = s*u
    (w = 1+w1 materialized on GPSIMD; t->bf16 cast on GPSIMD)
  - reductions: PE ones-matmuls accumulate per-image column sums in PSUM
  - host finishes from a tiny [4,3,512] stats tensor per core.
"""

import numpy as np

B, IMH, IMW = 32, 1024, 1024
NCORES = 8
IMGS = B // NCORES  # 4 images per core
NT = IMH // 128     # 8 row-tiles per image
NPAIR = NT // 2     # 4 pairs per image
PADW = 1072         # 31 zeros | 1024 data | 15 zeros | 2 unused (host-padded)
SMOOTH = 1e-3

_CACHE = {}


def _make_consts():
    k = np.arange(128)[:, None]  # lhsT row (contraction index within rhs block)
    m = np.arange(128)[None, :]  # lhsT col (output partition within out block)
    diag = (np.abs(m - k) <= 15).astype(np.float32)
    above = (np.abs(128 + m - k) <= 15).astype(np.float32)   # rhs = H_{i-1}
    below = (np.abs(-128 + m - k) <= 15).astype(np.float32)  # rhs = H_{i+1}
    wband = np.stack([diag, above, below])                   # [3,128,128]
    wident = (-961.0 * np.eye(128)).astype(np.float32)
    return wband, wident


def _build_program(reps=1, internal_inputs=False, stage=4):
    from contextlib import ExitStack

    import concourse.bacc as bacc
    import concourse.tile as tile
    from concourse import mybir

    f32 = mybir.dt.float32
    bf16 = mybir.dt.bfloat16
    AL = mybir.AluOpType
    AF = mybir.ActivationFunctionType

    nc = bacc.Bacc("TRN2", target_bir_lowering=False, debug=False)
    kind = "Internal" if internal_inputs else "ExternalInput"
    t_dram = nc.dram_tensor("target", [IMGS, IMH, PADW], f32, kind=kind)
    o_dram = nc.dram_tensor("outp", [IMGS, IMH, IMW], f32, kind=kind)
    wb_dram = nc.dram_tensor("wband", [3, 128, 128], f32, kind=kind)
    wi_dram = nc.dram_tensor("wident", [128, 128], f32, kind=kind)
    st_dram = nc.dram_tensor("stats", [IMGS, 3, 512], f32, kind="ExternalOutput")

    with tile.TileContext(nc) as tc, ExitStack() as ctx:
        consts = ctx.enter_context(tc.tile_pool(name="consts", bufs=1))
        tpool = ctx.enter_context(tc.tile_pool(name="tbuf", bufs=3))
        opool = ctx.enter_context(tc.tile_pool(name="obuf", bufs=3))
        hpool = ctx.enter_context(tc.tile_pool(name="hbuf", bufs=5))
        spool = ctx.enter_context(tc.tile_pool(name="sbuf16", bufs=3))
        t16pool = ctx.enter_context(tc.tile_pool(name="t16buf", bufs=3))
        prodpool = ctx.enter_context(tc.tile_pool(name="prod", bufs=2))
        psd = ctx.enter_context(tc.tile_pool(name="psd", bufs=2, space="PSUM"))
        psr = ctx.enter_context(tc.tile_pool(name="psr", bufs=2, space="PSUM"))
        stpool = ctx.enter_context(tc.tile_pool(name="stout", bufs=2))

        # ---- constants: load fp32 band blocks, cast to bf16 once ----
        wb_f = []
        wb16 = []
        for bi in range(3):
            wf = consts.tile([128, 128], f32, name=f"wbf{bi}", tag=f"wbf{bi}")
            nc.sync.dma_start(wf[:], wb_dram[bi])
            w16 = consts.tile([128, 128], bf16, name=f"wb16{bi}", tag=f"wb16{bi}")
            nc.vector.tensor_copy(w16[:], wf[:])
            wb_f.append(wf)
            wb16.append(w16)
        w_diag, w_above, w_below = wb16
        w_ident = consts.tile([128, 128], f32, tag="wident")
        nc.sync.dma_start(w_ident[:], wi_dram[:])
        ones16 = consts.tile([128, 1], bf16, tag="ones16")
        nc.vector.memset(ones16[:], 1.0)

        # ---- pipeline over (image, pair) with 1-pair lag ----
        seq = [(g, p) for _ in range(reps) for g in range(IMGS) for p in range(NPAIR)]
        tbs, hbs, sbs, t16s = {}, {}, {}, {}
        red = {}  # per-image psum accumulators

        def emit_load(idx, g, p):
            tb = tpool.tile([128, 2, PADW], f32, tag="tb")
            tsrc = t_dram[g, 256 * p : 256 * (p + 1), :].rearrange(
                "(two r) w -> r two w", two=2
            )
            nc.sync.dma_start(tb[:], tsrc)
            ob = opool.tile([128, 2, 1024], f32, tag="ob")
            osrc = o_dram[g, 256 * p : 256 * (p + 1), :].rearrange(
                "(two r) w -> r two w", two=2
            )
            nc.sync.dma_start(ob[:], osrc)

            if stage < 1:
                tbs[idx] = tb
                return
            hb = hpool.tile([128, 2, 1040], bf16, tag="hb")
            for k in range(2):
                # 15 warm-up steps accumulate t[0..14]; cols 15: are the box sums
                nc.vector.tensor_tensor_scan(
                    out=hb[:, k, 0:1039],
                    data0=tb[:, k, 31:1070],
                    data1=tb[:, k, 0:1039],
                    initial=0.0,
                    op0=AL.add,
                    op1=AL.subtract,
                )
            if stage < 2:
                tbs[idx], hbs[idx] = tb, hb
                return
            sb = spool.tile([128, 2, 1024], bf16, tag="sb")
            nc.scalar.activation(sb[:], ob[:], AF.Sigmoid)
            t16 = t16pool.tile([128, 2, 1024], bf16, tag="t16")
            nc.gpsimd.tensor_copy(t16[:], tb[:, :, 31:1055])
            tbs[idx], hbs[idx], sbs[idx], t16s[idx] = tb, hb, sb, t16

        def h_view(base, j, h):
            # H row-tile j (pair handle at seq index base + j//2 - p0), half h
            return hbs[base + j // 2][:, j % 2, 15 + 512 * h : 15 + 512 * (h + 1)]

        def emit_process(idx, g, p):
            if stage < 3:
                return
            base = idx - p  # seq index of this image's pair 0
            tb = tbs[idx]
            w1 = prodpool.tile([128, 2, 1024], bf16, tag="w1")
            for k in range(2):
                j = 2 * p + k
                for h in range(2):
                    dps = psd.tile([128, 512], f32, tag="dps")
                    nc.tensor.matmul(
                        dps[:], w_diag[:], h_view(base, j, h), start=True, stop=False
                    )
                    if j > 0:
                        nc.tensor.matmul(
                            dps[:], w_above[:], h_view(base, j - 1, h),
                            start=False, stop=False,
                        )
                    if j < NT - 1:
                        nc.tensor.matmul(
                            dps[:], w_below[:], h_view(base, j + 1, h),
                            start=False, stop=False,
                        )
                    nc.tensor.matmul(
                        dps[:], w_ident[:],
                        tb[:, k, 31 + 512 * h : 31 + 512 * (h + 1)],
                        start=False, stop=True,
                    )
                    nc.scalar.activation(
                        out=w1[:, k, 512 * h : 512 * (h + 1)],
                        in_=dps[:],
                        func=AF.Abs,
                        scale=5.0 / 961.0,
                    )
            if stage < 4:
                return
            w1f = w1[:].rearrange("p a b -> p (a b)")
            wt = prodpool.tile([128, 2048], bf16, tag="wt")
            nc.gpsimd.tensor_scalar_add(wt[:], w1f, 1.0)
            t16f = t16s[idx][:].rearrange("p a b -> p (a b)")
            sbf = sbs[idx][:].rearrange("p a b -> p (a b)")
            u = prodpool.tile([128, 2048], bf16, tag="u")
            nc.vector.tensor_mul(u[:], t16f, wt[:])
            v = prodpool.tile([128, 2048], bf16, tag="v")
            nc.vector.tensor_mul(v[:], sbf, wt[:])
            p2 = prodpool.tile([128, 2048], bf16, tag="p2")
            nc.vector.tensor_mul(p2[:], sbf, u[:])

            if p == 0:
                red[g] = [psr.tile([1, 512], f32, name=f"red{q}", tag=f"red{q}") for q in range(3)]
            for q, src in enumerate((u, v, p2)):
                for c in range(4):
                    nc.tensor.matmul(
                        red[g][q][:],
                        ones16[:],
                        src[:, 512 * c : 512 * (c + 1)],
                        start=(p == 0 and c == 0 and q >= 0 and True),
                        stop=(p == NPAIR - 1 and c == 3),
                        skip_group_check=True,
                    )

        def emit_evac(g):
            st = stpool.tile([1, 3, 512], f32, tag="st")
            for q in range(3):
                nc.scalar.copy(st[:, q, :], red[g][q][:])
            nc.sync.dma_start(st_dram[g : g + 1], st[:])

        for idx in range(len(seq) + 1):
            if idx < len(seq):
                emit_load(idx, *seq[idx])
            if idx >= 1:
                g, p = seq[idx - 1]
                emit_process(idx - 1, g, p)
                if p == NPAIR - 1 and stage >= 4:
                    emit_evac(g)

    nc.compile()
    return nc


def _get_program(reps=1, internal_inputs=False, stage=4):
    key = ("nc", reps, internal_inputs, stage)
    if key not in _CACHE:
        _CACHE[key] = _build_program(reps, internal_inputs, stage)
    return _CACHE[key]


def run_on_device(in_maps, **kwargs):
    from concourse.bass_utils import run_bass_kernel_spmd

    nc = _get_program()
    return run_bass_kernel_spmd(nc, in_maps, core_ids=list(range(NCORES)), **kwargs)


def make_in_maps(output, target):
    output = np.asarray(output, dtype=np.float32)
    target = np.asarray(target, dtype=np.float32)
    wband, wident = _make_consts()
    in_maps = []
    tpad = np.zeros((B, IMH, PADW), dtype=np.float32)
    tpad[:, :, 31:1055] = target[:, 0]
    for c in range(NCORES):
        in_maps.append(
            {
                "target": tpad[c * IMGS : (c + 1) * IMGS],
                "outp": np.ascontiguousarray(output[c * IMGS : (c + 1) * IMGS, 0]),
                "wband": wband,
                "wident": wident,
            }
        )
    return in_maps


def finish_on_host(results):
    losses = []
    for c in range(NCORES):
        st = np.asarray(results[c]["stats"], dtype=np.float64)  # [IMGS,3,512]
        for g in range(IMGS):
            su = st[g, 0].sum()
            sv = st[g, 1].sum()
            sp2 = st[g, 2].sum()
            inter = sp2
            mask = su + sv
            losses.append(1.0 - (2.0 * inter + SMOOTH) / (mask + SMOOTH))
    return np.float32(np.mean(losses))


def kernel(output, target):
    in_maps = make_in_maps(output, target)
    res = run_on_device(in_maps)
    return finish_on_host(res.results)



# revision 4
# speedup vs baseline: 3.7771x; 3.7771x over previous
"""DiceLoss kernel for Trainium2 (8 NeuronCores, batch-sharded).

Math per image (H=W=1024):
    s  = sigmoid(output)
    P  = avgpool31x31(target)            (zero-padded box sum / 961)
    w  = 1 + 5*|P - target|
    inter = sum(s*t*w);  mask = sum((t+s)*w)
    loss  = 1 - (2*inter + 1e-3) / (mask + 1e-3);  final = mean over batch

Device strategy (per core: 4 images, 8 row-tiles of [128, 1024] each,
processed in pairs of row-tiles = [128, 2048] working set). All inputs
arrive as bf16 (host casts), halving HBM traffic:
  - horizontal 31-tap box: ONE tensor_tensor_scan per row-tile
        state = (t[x+15] + state) - t[x-16]   (fp32 state) over padded rows
  - vertical 31-tap box and the "- 961*t" subtraction: PE band matmuls
        psum = Bdiag@H_i + Babove@H_{i-1} + Bbelow@H_{i+1}
               - 960*I@t_i - 1*I@t_i        (both exact in bf16)
  - w1 = |5/961 * psum| on ACT (scale fused into Abs), bf16 out
  - fused DVE products with built-in row reductions (accum_out):
        u  = (w1+1)*t   with accum -> sum(t*w)    [scalar_tensor_tensor]
        v  = (w1+1)*s   with accum -> sum(s*w)    [scalar_tensor_tensor]
        p2 = s*u        with accum -> sum(s*t*w)  [tensor_tensor_reduce]
  - per-pair per-partition sums land in a [128, 48] f32 stash; host does
    the final cross-partition/cross-pair sums and the loss arithmetic.
No GPSIMD ops (they run ~25x below spec and lock the SBUF port shared
with DVE), no on-device casts, no PE reduction matmuls.
"""

import numpy as np

B, IMH, IMW = 32, 1024, 1024
NCORES = 8
IMGS = B // NCORES  # 4 images per core
NT = IMH // 128     # 8 row-tiles per image
NPAIR = NT // 2     # 4 pairs per image
PADW = 1072         # 31 zeros | 1024 data | 15 zeros | 2 unused (host-padded)
SMOOTH = 1e-3

_CACHE = {}


def _make_consts():
    k = np.arange(128)[:, None]  # lhsT row (contraction index within rhs block)
    m = np.arange(128)[None, :]  # lhsT col (output partition within out block)
    diag = (np.abs(m - k) <= 15).astype(np.float32)
    above = (np.abs(128 + m - k) <= 15).astype(np.float32)   # rhs = H_{i-1}
    below = (np.abs(-128 + m - k) <= 15).astype(np.float32)  # rhs = H_{i+1}
    wband = np.stack([diag, above, below])                   # [3,128,128]
    wident = np.stack(
        [-960.0 * np.eye(128), -1.0 * np.eye(128)]
    ).astype(np.float32)                                     # [2,128,128]
    return wband, wident


def _build_program():
    from contextlib import ExitStack

    import concourse.bacc as bacc
    import concourse.tile as tile
    from concourse import mybir

    f32 = mybir.dt.float32
    bf16 = mybir.dt.bfloat16
    AL = mybir.AluOpType
    AF = mybir.ActivationFunctionType

    nc = bacc.Bacc("TRN2", target_bir_lowering=False, debug=False)
    t_dram = nc.dram_tensor("target", [IMGS, IMH, PADW], bf16, kind="ExternalInput")
    o_dram = nc.dram_tensor("outp", [IMGS, IMH, IMW], bf16, kind="ExternalInput")
    wb_dram = nc.dram_tensor("wband", [3, 128, 128], bf16, kind="ExternalInput")
    wi_dram = nc.dram_tensor("wident", [2, 128, 128], bf16, kind="ExternalInput")
    st_dram = nc.dram_tensor("stats", [128, IMGS * NPAIR * 3], f32, kind="ExternalOutput")

    with tile.TileContext(nc) as tc, ExitStack() as ctx:
        consts = ctx.enter_context(tc.tile_pool(name="consts", bufs=1))
        tpool = ctx.enter_context(tc.tile_pool(name="tbuf", bufs=3))
        opool = ctx.enter_context(tc.tile_pool(name="obuf", bufs=3))
        hpool = ctx.enter_context(tc.tile_pool(name="hbuf", bufs=5))
        spool = ctx.enter_context(tc.tile_pool(name="sbuf16", bufs=3))
        w1pool = ctx.enter_context(tc.tile_pool(name="w1buf", bufs=2))
        upool = ctx.enter_context(tc.tile_pool(name="ubuf", bufs=2))
        junkpool = ctx.enter_context(tc.tile_pool(name="junk", bufs=2))
        psd = ctx.enter_context(tc.tile_pool(name="psd", bufs=6, space="PSUM"))
        stpool = ctx.enter_context(tc.tile_pool(name="stout", bufs=1))

        wband = consts.tile([128, 3, 128], bf16, tag="wband")
        nc.sync.dma_start(wband[:], wb_dram.rearrange("a b c -> b a c"))
        wident = consts.tile([128, 2, 128], bf16, tag="wident")
        nc.sync.dma_start(wident[:], wi_dram.rearrange("a b c -> b a c"))

        stash = stpool.tile([128, IMGS * NPAIR, 3], f32, tag="stash")

        # ---- pipeline over (image, pair) with 1-pair lag ----
        seq = [(g, p) for g in range(IMGS) for p in range(NPAIR)]
        tbs, hbs, sbs = {}, {}, {}

        def emit_load(idx, g, p):
            tb = tpool.tile([128, 2, PADW], bf16, tag="tb")
            tsrc = t_dram[g, 256 * p : 256 * (p + 1), :].rearrange(
                "(two r) w -> r two w", two=2
            )
            nc.sync.dma_start(tb[:], tsrc)
            ob = opool.tile([128, 2, 1024], bf16, tag="ob")
            osrc = o_dram[g, 256 * p : 256 * (p + 1), :].rearrange(
                "(two r) w -> r two w", two=2
            )
            nc.sync.dma_start(ob[:], osrc)

            hb = hpool.tile([128, 2, 1040], bf16, tag="hb")
            for k in range(2):
                # 15 warm-up steps accumulate t[0..14]; cols 15: are box sums
                nc.vector.tensor_tensor_scan(
                    out=hb[:, k, 0:1039],
                    data0=tb[:, k, 31:1070],
                    data1=tb[:, k, 0:1039],
                    initial=0.0,
                    op0=AL.add,
                    op1=AL.subtract,
                )
            sb = spool.tile([128, 2, 1024], bf16, tag="sb")
            nc.scalar.activation(sb[:], ob[:], AF.Sigmoid)
            tbs[idx], hbs[idx], sbs[idx] = tb, hb, sb

        def h_view(base, j, h):
            # H row-tile j (pair handle at seq index base + j//2), half h
            return hbs[base + j // 2][:, j % 2, 15 + 512 * h : 15 + 512 * (h + 1)]

        def emit_process(idx, g, p):
            base = idx - p  # seq index of this image's pair 0
            tb = tbs[idx]
            sb = sbs[idx]
            w1 = w1pool.tile([128, 2, 1024], bf16, tag="w1")
            for k in range(2):
                j = 2 * p + k
                for h in range(2):
                    dps = psd.tile([128, 512], f32, tag="dps")
                    nc.tensor.matmul(
                        dps[:], wband[:, 0, :], h_view(base, j, h),
                        start=True, stop=False,
                    )
                    if j > 0:
                        nc.tensor.matmul(
                            dps[:], wband[:, 1, :], h_view(base, j - 1, h),
                            start=False, stop=False,
                        )
                    if j < NT - 1:
                        nc.tensor.matmul(
                            dps[:], wband[:, 2, :], h_view(base, j + 1, h),
                            start=False, stop=False,
                        )
                    tview = tb[:, k, 31 + 512 * h : 31 + 512 * (h + 1)]
                    nc.tensor.matmul(
                        dps[:], wident[:, 0, :], tview, start=False, stop=False
                    )
                    nc.tensor.matmul(
                        dps[:], wident[:, 1, :], tview, start=False, stop=True
                    )
                    nc.scalar.activation(
                        out=w1[:, k, 512 * h : 512 * (h + 1)],
                        in_=dps[:],
                        func=AF.Abs,
                        scale=5.0 / 961.0,
                    )

            pair = g * NPAIR + p
            tf = tb[:, :, 31:1055]
            u = upool.tile([128, 2, 1024], bf16, tag="u")
            nc.vector.scalar_tensor_tensor(
                out=u[:],
                in0=w1[:], scalar=1.0, in1=tf,
                op0=AL.add, op1=AL.mult,
                accum_out=stash[:, pair, 0:1],
            )
            v = junkpool.tile([128, 2, 1024], bf16, tag="v")
            nc.vector.scalar_tensor_tensor(
                out=v[:],
                in0=w1[:], scalar=1.0, in1=sb[:],
                op0=AL.add, op1=AL.mult,
                accum_out=stash[:, pair, 1:2],
            )
            p2 = junkpool.tile([128, 2, 1024], bf16, tag="p2")
            nc.vector.scalar_tensor_tensor(
                out=p2[:],
                in0=sb[:], scalar=1.0, in1=u[:],
                op0=AL.mult, op1=AL.mult,
                accum_out=stash[:, pair, 2:3],
            )

        for idx in range(len(seq) + 1):
            if idx < len(seq):
                emit_load(idx, *seq[idx])
            if idx >= 1:
                emit_process(idx - 1, *seq[idx - 1])

        nc.sync.dma_start(
            st_dram[:], stash[:].rearrange("p a b -> p (a b)")
        )

    nc.compile()
    return nc


def _get_program():
    key = "nc"
    if key not in _CACHE:
        _CACHE[key] = _build_program()
    return _CACHE[key]


def run_on_device(in_maps, **kwargs):
    from concourse.bass_utils import run_bass_kernel_spmd

    nc = _get_program()
    return run_bass_kernel_spmd(nc, in_maps, core_ids=list(range(NCORES)), **kwargs)


def make_in_maps(output, target):
    import ml_dtypes

    bf16 = ml_dtypes.bfloat16
    output = np.asarray(output, dtype=np.float32)
    target = np.asarray(target, dtype=np.float32)
    wband, wident = _make_consts()
    in_maps = []
    tpad = np.zeros((B, IMH, PADW), dtype=bf16)
    tpad[:, :, 31:1055] = target[:, 0].astype(bf16)
    o16 = output[:, 0].astype(bf16)
    wband16 = wband.astype(bf16)
    wident16 = wident.astype(bf16)
    for c in range(NCORES):
        in_maps.append(
            {
                "target": tpad[c * IMGS : (c + 1) * IMGS],
                "outp": np.ascontiguousarray(o16[c * IMGS : (c + 1) * IMGS]),
                "wband": wband16,
                "wident": wident16,
            }
        )
    return in_maps


def finish_on_host(results):
    losses = []
    for c in range(NCORES):
        st = np.asarray(results[c]["stats"], dtype=np.float64)  # [128, 48]
        st = st.reshape(128, IMGS, NPAIR, 3).sum(axis=(0, 2))   # [IMGS, 3]
        for g in range(IMGS):
            su, sv, sp2 = st[g]
            inter = sp2
            mask = su + sv
            losses.append(1.0 - (2.0 * inter + SMOOTH) / (mask + SMOOTH))
    return np.float32(np.mean(losses))


def kernel(output, target):
    in_maps = make_in_maps(output, target)
    res = run_on_device(in_maps)
    return finish_on_host(res.results)
